# revision 5
# baseline (speedup 1.0000x reference)
"""Causal self-attention (QKV proj + RoPE + causal SDPA + out proj) on 8 TRN2 cores.

Sharding: 8 cores = 4 batches x 2 head-groups (tensor-parallel over heads).
Core c handles batch b=c//2, head group g=c%2 (8 of 16 heads). Each core:
  - QKV.T projection with head_dim on partitions (f32r matmuls, 1 cyc/row)
  - RoPE applied in a de-interleaved basis (even components in partitions 0:64,
    odd in 64:128) so the rotation pair sits at partition offset +64; the
    de-interleave is folded into the host-side weight row ordering.
  - causal attention computed transposed: S.T[k,q] tiles so exp'd scores feed
    the AV matmul directly as rhs (no P transpose); softmax denominator via
    M=1 ones-matmul accumulated in PSUM; normalize via gpsimd
    partition_broadcast of the reciprocal row.
  - out projection accumulated over the 8 local head chunks -> partial out.T.
Host sums the two tensor-parallel partials per batch and reassembles k/v.

Returns (out, k, v) matching the reference: out [B,S,D], k/v [B,H,S,hd]
(k is post-RoPE, v pre-RoPE), all float32.
"""

import sys

if "/opt/trn_rl_repo" not in sys.path:
    sys.path.insert(0, "/opt/trn_rl_repo")

import math

import numpy as np

import concourse.bacc as bacc
import concourse.mybir as mybir
from concourse import tile
from concourse.bass_utils import run_bass_kernel_spmd

F32 = mybir.dt.float32
F32R = mybir.dt.float32r
EXPF = mybir.ActivationFunctionType.Exp

N_CORES = 8


class Cfg:
    def __init__(self, d_model=2048, s=2048, n_heads=16):
        self.D = d_model
        self.S = s
        self.H = n_heads                  # total heads
        self.HD = 128                     # head dim (fixed: masks/rope assume 128)
        self.HPC = n_heads // 2           # heads per core (2-way head TP)
        self.DC = d_model // 128          # contraction chunks
        self.SB = max(1, s // 512)        # 512-wide s blocks
        self.SBW = min(512, s)            # s block width
        self.ST = s // 128                # 128-row s tiles
        self.EV = self.HPC * 128          # v cols per core
        self.EVB = max(1, self.EV // 512) # v col blocks
        self.EVW = min(512, self.EV)      # v col block width


def build(cfg: Cfg):
    """Build the per-core Bass program (identical across cores; data differs)."""
    nc = bacc.Bacc("TRN2", target_bir_lowering=False, debug=False)
    D, S, HPC, DC, SB, SBW, ST = cfg.D, cfg.S, cfg.HPC, cfg.DC, cfg.SB, cfg.SBW, cfg.ST
    EV, EVB, EVW = cfg.EV, cfg.EVB, cfg.EVW
    QB = SB                     # 512-wide q blocks
    KTPB = SBW // 128           # k tiles per q block width

    xt_d = nc.dram_tensor("xt", [D, S], F32R, kind="ExternalInput")
    wq_d = nc.dram_tensor("wq", [D, HPC * 128], F32R, kind="ExternalInput")
    wk_d = nc.dram_tensor("wk", [D, HPC * 128], F32R, kind="ExternalInput")
    wv_d = nc.dram_tensor("wv", [D, EV], F32R, kind="ExternalInput")
    wo_d = nc.dram_tensor("wo", [EV, D], F32R, kind="ExternalInput")
    ropec_d = nc.dram_tensor("ropec", [128, S], F32, kind="ExternalInput")
    ropes2_d = nc.dram_tensor("ropes2", [128, S], F32, kind="ExternalInput")
    maskt_d = nc.dram_tensor("maskt", [128, 128], F32, kind="ExternalInput")
    ones_d = nc.dram_tensor("ones1", [128, 1], F32R, kind="ExternalInput")

    out_d = nc.dram_tensor("out_t", [D, S], F32, kind="ExternalOutput")
    k_d = nc.dram_tensor("k_out", [HPC, 128, S], F32R, kind="ExternalOutput")
    v_d = nc.dram_tensor("v_out", [S, EV], F32R, kind="ExternalOutput")
    q_scr = nc.dram_tensor("q_scr", [HPC, 128, S], F32R)
    ctx_scr = nc.dram_tensor("ctx_scr", [HPC, 128, S], F32R)

    inv_sqrt_hd = 1.0 / math.sqrt(128.0)

    with tile.TileContext(nc) as tc:
        with tc.tile_pool(name="small", bufs=1) as small:
            maskt = small.tile([128, 128], F32)
            nc.sync.dma_start(maskt[:], maskt_d[:])
            ones = small.tile([128, 1], F32R)
            nc.sync.dma_start(ones[:], ones_d[:])

            # ---------------- Phase P: projections (+ RoPE for q/k) ----------
            with (
                tc.tile_pool(name="xt", bufs=1) as xtp,
                tc.tile_pool(name="tbl", bufs=1) as tblp,
                tc.tile_pool(name="whd", bufs=2) as whdp,
                tc.tile_pool(name="wvt", bufs=3) as wvtp,
                tc.tile_pool(name="rtmp", bufs=3) as rtmpp,
                tc.tile_pool(name="rot", bufs=3) as rotp,
                tc.tile_pool(name="vev", bufs=3) as vevp,
                tc.tile_pool(name="pps", bufs=3, space="PSUM") as pps,
                tc.tile_pool(name="vps", bufs=1, space="PSUM") as vps,
            ):
                xt = []
                for d in range(DC):
                    t = xtp.tile([128, S], F32R, tag=f"xt{d}")
                    nc.sync.dma_start(t[:], xt_d[d * 128:(d + 1) * 128, :])
                    xt.append(t)
                ropec = tblp.tile([128, S], F32, tag="ropec")
                nc.sync.dma_start(ropec[:], ropec_d[:])
                ropes2 = tblp.tile([128, S], F32, tag="ropes2")
                nc.sync.dma_start(ropes2[:], ropes2_d[:])

                def project_rope(w_dram, h, dst):
                    """Project one q/k head tile [128, S] with RoPE, spill to dst[h]."""
                    whd = whdp.tile([128, DC, 128], F32R, tag="whd")
                    src = w_dram[:, h * 128:(h + 1) * 128].rearrange(
                        "(d p) e -> p d e", p=128)
                    nc.sync.dma_start(whd[:], src)
                    for s in range(SB):
                        sl = slice(s * SBW, (s + 1) * SBW)
                        ps = pps.tile([128, SBW], F32, tag="proj")
                        for d in range(DC):
                            nc.tensor.matmul(ps[:], whd[:, d, :], xt[d][:, sl],
                                             start=(d == 0), stop=(d == DC - 1))
                        # RoPE: rot = ps*C + swap64(ps)*S2  (swap via psum read
                        # at partition offset 64 -- legal since in0 is PSUM)
                        m1 = rtmpp.tile([128, SBW], F32, tag="m1")
                        nc.vector.tensor_mul(m1[:], ps[:], ropec[:, sl])
                        m2 = rtmpp.tile([128, SBW], F32, tag="m2")
                        nc.vector.tensor_mul(m2[0:64, :], ps[64:128, :],
                                             ropes2[0:64, sl])
                        nc.vector.tensor_mul(m2[64:128, :], ps[0:64, :],
                                             ropes2[64:128, sl])
                        rot = rotp.tile([128, SBW], F32R, tag="rot")
                        nc.vector.tensor_add(rot[:], m1[:], m2[:])
                        nc.sync.dma_start(dst[h, :, s * SBW:(s + 1) * SBW], rot[:])

                for h in range(HPC):
                    project_rope(wk_d, h, k_d)
                for h in range(HPC):
                    project_rope(wq_d, h, q_scr)

                # V: natural layout out[s, e] = sum_d x[s, d] Wv[e, d]
                # lhsT = xt tile column slice (stationary), rhs = wv chunk.
                for eb in range(EVB):
                    ebl = slice(eb * EVW, (eb + 1) * EVW)
                    for stg in range(0, ST, 4):
                        n_st = min(4, ST - stg)
                        pss = []
                        for i in range(n_st):
                            vp = vps.tile([128, EVW], F32, tag=f"vp{i}",
                                          name=f"vp{i}")
                            pss.append(vp)
                        for d in range(DC):
                            wvt = wvtp.tile([128, EVW], F32R, tag="wvt")
                            nc.sync.dma_start(
                                wvt[:], wv_d[d * 128:(d + 1) * 128, ebl])
                            for i in range(n_st):
                                st = stg + i
                                nc.tensor.matmul(
                                    pss[i][:], xt[d][:, st * 128:(st + 1) * 128],
                                    wvt[:], start=(d == 0), stop=(d == DC - 1))
                        for i in range(n_st):
                            st = stg + i
                            vt = vevp.tile([128, EVW], F32R, tag="vev")
                            nc.vector.tensor_copy(vt[:], pss[i][:])
                            nc.sync.dma_start(
                                v_d[st * 128:(st + 1) * 128, ebl], vt[:])

            # ---------------- Phase A: causal attention ----------------------
            with (
                tc.tile_pool(name="qkv", bufs=2) as qkvp,
                tc.tile_pool(name="exp", bufs=6) as expp,
                tc.tile_pool(name="bcst", bufs=3) as bcp,
                tc.tile_pool(name="ctxs", bufs=3) as ctxp,
                tc.tile_pool(name="qkps", bufs=3, space="PSUM") as qkps,
                tc.tile_pool(name="ctxps", bufs=2, space="PSUM") as ctxps,
                tc.tile_pool(name="denps", bufs=2, space="PSUM") as denps,
            ):
                for h in range(HPC):
                    qT = qkvp.tile([128, S], F32R, tag="qT")
                    nc.sync.dma_start(qT[:], q_scr[h])
                    kT = qkvp.tile([128, S], F32R, tag="kT")
                    nc.sync.dma_start(kT[:], k_d[h])
                    vT = qkvp.tile([128, ST, 128], F32R, tag="vT")
                    nc.sync.dma_start(
                        vT[:], v_d[:, h * 128:(h + 1) * 128].rearrange(
                            "(c p) e -> p c e", p=128))
                    for j in range(QB):
                        q0 = j * SBW
                        n_k = KTPB * j + KTPB
                        ctx_ps = ctxps.tile([128, SBW], F32, tag="ctx")
                        den_ps = denps.tile([1, SBW], F32, tag="den")
                        for i in range(n_k):
                            delta = max(0, (i - KTPB * j) * 128)
                            w = SBW - delta
                            s_ps = qkps.tile([128, SBW], F32, tag="qk")
                            nc.tensor.matmul(
                                s_ps[:, 0:w], kT[:, i * 128:(i + 1) * 128],
                                qT[:, q0 + delta:q0 + SBW],
                                start=True, stop=True)
                            e_sb = expp.tile([128, SBW], F32R, tag="exp")
                            nc.scalar.activation(e_sb[:, 0:w], s_ps[:, 0:w],
                                                 EXPF, scale=inv_sqrt_hd)
                            if i >= KTPB * j:
                                # diagonal tile: triangular causal mask on the
                                # first 128 columns of the restricted range
                                nc.vector.tensor_mul(e_sb[:, 0:128],
                                                     e_sb[:, 0:128], maskt[:])
                            nc.tensor.matmul(
                                ctx_ps[:, delta:SBW], vT[:, i, :], e_sb[:, 0:w],
                                start=(i == 0), stop=(i == n_k - 1))
                            nc.tensor.matmul(
                                den_ps[0:1, delta:SBW], ones[:], e_sb[:, 0:w],
                                start=(i == 0), stop=(i == n_k - 1))
                        recip = bcp.tile([1, SBW], F32, tag="recip")
                        nc.vector.reciprocal(recip[:], den_ps[:])
                        bc = bcp.tile([128, SBW], F32, tag="bc")
                        nc.gpsimd.partition_broadcast(bc[:], recip[:])
                        ctx_sb = ctxp.tile([128, SBW], F32R, tag="ctxsb")
                        nc.vector.tensor_mul(ctx_sb[:], ctx_ps[:], bc[:])
                        nc.sync.dma_start(
                            ctx_scr[h, :, q0:q0 + SBW], ctx_sb[:])

            # ---------------- Phase O: output projection ---------------------
            with (
                tc.tile_pool(name="wo", bufs=1) as wop,
                tc.tile_pool(name="ctxsl", bufs=2) as cslp,
                tc.tile_pool(name="osb", bufs=4) as osbp,
                tc.tile_pool(name="ops", bufs=4, space="PSUM") as ops,
            ):
                wo = []
                for h in range(HPC):
                    t = wop.tile([128, D], F32R, tag=f"wo{h}")
                    nc.sync.dma_start(t[:], wo_d[h * 128:(h + 1) * 128, :])
                    wo.append(t)
                for s in range(SB):
                    sl = slice(s * SBW, (s + 1) * SBW)
                    csl = []
                    for h in range(HPC):
                        t = cslp.tile([128, SBW], F32R, tag=f"csl{h}")
                        nc.sync.dma_start(t[:], ctx_scr[h, :, sl])
                        csl.append(t)
                    for m in range(D // 128):
                        ps = ops.tile([128, SBW], F32, tag="ops")
                        for h in range(HPC):
                            nc.tensor.matmul(
                                ps[:], wo[h][:, m * 128:(m + 1) * 128], csl[h][:],
                                start=(h == 0), stop=(h == HPC - 1))
                        ot = osbp.tile([128, SBW], F32, tag="osb")
                        nc.vector.tensor_copy(ot[:], ps[:])
                        nc.sync.dma_start(out_d[m * 128:(m + 1) * 128, sl], ot[:])

    nc.compile()
    return nc


def host_inputs(cfg: Cfg, x, rope_cos, rope_sin, qkv_w, out_w):
    """Build the 8 per-core input maps from full inputs."""
    D, S, HPC = cfg.D, cfg.S, cfg.HPC
    deint = np.concatenate([np.arange(0, 128, 2), np.arange(1, 128, 2)])
    ropec = np.concatenate([rope_cos.T, rope_cos.T], axis=0).astype(np.float32)
    ropec = np.ascontiguousarray(ropec)
    ropes2 = np.concatenate([-rope_sin.T, rope_sin.T], axis=0).astype(np.float32)
    ropes2 = np.ascontiguousarray(ropes2)
    kk, qq = np.meshgrid(np.arange(128), np.arange(128), indexing="ij")
    maskt = (kk <= qq).astype(np.float32)

    in_maps = []
    for c in range(N_CORES):
        b, g = c // 2, c % 2
        xt = np.ascontiguousarray(x[b].T)
        rows_q = np.concatenate(
            [(g * HPC + h) * 128 + deint for h in range(HPC)])
        rows_k = D + rows_q
        rows_v = np.concatenate(
            [2 * D + (g * HPC + h) * 128 + np.arange(128) for h in range(HPC)])
        wq = np.ascontiguousarray(qkv_w[rows_q].T)
        wk = np.ascontiguousarray(qkv_w[rows_k].T)
        wv = np.ascontiguousarray(qkv_w[rows_v].T)
        wo = np.ascontiguousarray(out_w[:, g * HPC * 128:(g + 1) * HPC * 128].T)
        in_maps.append({
            "xt": xt, "wq": wq, "wk": wk, "wv": wv, "wo": wo,
            "ropec": ropec, "ropes2": ropes2, "maskt": maskt,
            "ones1": np.ones((128, 1), dtype=np.float32),
        })
    return in_maps


def assemble(cfg: Cfg, results):
    """Gather per-core results into full (out, k, v)."""
    D, S, HPC = cfg.D, cfg.S, cfg.HPC
    B, H = 4, 2 * HPC
    deint = np.concatenate([np.arange(0, 128, 2), np.arange(1, 128, 2)])
    out = np.empty((B, S, D), dtype=np.float32)
    k = np.empty((B, H, S, 128), dtype=np.float32)
    v = np.empty((B, H, S, 128), dtype=np.float32)
    for b in range(B):
        r0, r1 = results[2 * b], results[2 * b + 1]
        out[b] = (r0["out_t"] + r1["out_t"]).T
        for g, r in ((0, r0), (1, r1)):
            for h in range(HPC):
                k[b, g * HPC + h][:, deint] = r["k_out"][h].T
                v[b, g * HPC + h] = r["v_out"][:, h * 128:(h + 1) * 128]
    return out, k, v


_CACHE = {}


def _get_nc(cfg: Cfg):
    key = (cfg.D, cfg.S, cfg.H)
    if key not in _CACHE:
        _CACHE[key] = build(cfg)
    return _CACHE[key]


def run(inputs, cfg: Cfg | None = None, trace=False):
    cfg = cfg or Cfg()
    nc = _get_nc(cfg)
    in_maps = host_inputs(cfg, inputs["x"], inputs["rope_cos"],
                          inputs["rope_sin"], inputs["qkv_w"], inputs["out_w"])
    res = run_bass_kernel_spmd(nc, in_maps, list(range(N_CORES)), trace=trace)
    return assemble(cfg, res.results), res


def kernel(x, rope_cos, rope_sin, qkv_w, out_w):
    (out, k, v), _ = run({"x": np.asarray(x, dtype=np.float32),
                          "rope_cos": np.asarray(rope_cos, dtype=np.float32),
                          "rope_sin": np.asarray(rope_sin, dtype=np.float32),
                          "qkv_w": np.asarray(qkv_w, dtype=np.float32),
                          "out_w": np.asarray(out_w, dtype=np.float32)})
    return out, k, v


# revision 6
# speedup vs baseline: 1.0444x; 1.0444x over previous
"""Causal self-attention (QKV proj + RoPE + causal SDPA + out proj) on 8 TRN2 cores.

Sharding: 8 cores = 4 batches x 2 head-groups (tensor-parallel over heads).
Core c handles batch b=c//2, head group g=c%2 (8 of 16 heads). Each core:
  - QKV.T projection with head_dim on partitions (f32r matmuls, 1 cyc/row)
  - RoPE applied in a de-interleaved basis (even components in partitions 0:64,
    odd in 64:128) so the rotation pair sits at partition offset +64; the
    de-interleave is folded into the host-side weight row ordering.
  - causal attention computed transposed: S.T[k,q] tiles so exp'd scores feed
    the AV matmul directly as rhs (no P transpose); softmax denominator via
    M=1 ones-matmul accumulated in PSUM; normalize via gpsimd
    partition_broadcast of the reciprocal row.
  - out projection accumulated over the 8 local head chunks -> partial out.T.
Host sums the two tensor-parallel partials per batch and reassembles k/v.

Returns (out, k, v) matching the reference: out [B,S,D], k/v [B,H,S,hd]
(k is post-RoPE, v pre-RoPE), all float32.
"""

import sys

if "/opt/trn_rl_repo" not in sys.path:
    sys.path.insert(0, "/opt/trn_rl_repo")

import math

import numpy as np

import concourse.bacc as bacc
import concourse.mybir as mybir
from concourse import tile
from concourse.bass_utils import run_bass_kernel_spmd

F32 = mybir.dt.float32
F32R = mybir.dt.float32r
EXPF = mybir.ActivationFunctionType.Exp

N_CORES = 8


class Cfg:
    def __init__(self, d_model=2048, s=2048, n_heads=16):
        self.D = d_model
        self.S = s
        self.H = n_heads                  # total heads
        self.HD = 128                     # head dim (fixed: masks/rope assume 128)
        self.HPC = n_heads // 2           # heads per core (2-way head TP)
        self.DC = d_model // 128          # contraction chunks
        self.SB = max(1, s // 512)        # 512-wide s blocks
        self.SBW = min(512, s)            # s block width
        self.ST = s // 128                # 128-row s tiles
        self.EV = self.HPC * 128          # v cols per core
        self.EVB = max(1, self.EV // 512) # v col blocks
        self.EVW = min(512, self.EV)      # v col block width


def build(cfg: Cfg):
    """Build the per-core Bass program (identical across cores; data differs)."""
    nc = bacc.Bacc("TRN2", target_bir_lowering=False, debug=False)
    D, S, HPC, DC, SB, SBW, ST = cfg.D, cfg.S, cfg.HPC, cfg.DC, cfg.SB, cfg.SBW, cfg.ST
    EV, EVB, EVW = cfg.EV, cfg.EVB, cfg.EVW
    QB = SB                     # 512-wide q blocks
    KTPB = SBW // 128           # k tiles per q block width

    xt_d = nc.dram_tensor("xt", [D, S], F32R, kind="ExternalInput")
    wq_d = nc.dram_tensor("wq", [D, HPC * 128], F32R, kind="ExternalInput")
    wk_d = nc.dram_tensor("wk", [D, HPC * 128], F32R, kind="ExternalInput")
    wv_d = nc.dram_tensor("wv", [D, EV], F32R, kind="ExternalInput")
    wo_d = nc.dram_tensor("wo", [EV, D], F32R, kind="ExternalInput")
    ropec_d = nc.dram_tensor("ropec", [128, S], F32, kind="ExternalInput")
    ropes2_d = nc.dram_tensor("ropes2", [128, S], F32, kind="ExternalInput")
    maskt_d = nc.dram_tensor("maskt", [128, 128], F32, kind="ExternalInput")
    ones_d = nc.dram_tensor("ones1", [128, 1], F32R, kind="ExternalInput")

    out_d = nc.dram_tensor("out_t", [D, S], F32, kind="ExternalOutput")
    k_d = nc.dram_tensor("k_out", [HPC, 128, S], F32R, kind="ExternalOutput")
    v_d = nc.dram_tensor("v_out", [S, EV], F32R, kind="ExternalOutput")
    q_scr = nc.dram_tensor("q_scr", [HPC, 128, S], F32R)
    ctx_scr = nc.dram_tensor("ctx_scr", [HPC, 128, S], F32R)

    inv_sqrt_hd = 1.0 / math.sqrt(128.0)

    with tile.TileContext(nc) as tc:
        with tc.tile_pool(name="small", bufs=1) as small:
            maskt = small.tile([128, 128], F32)
            nc.sync.dma_start(maskt[:], maskt_d[:])
            ones = small.tile([128, 1], F32R)
            nc.sync.dma_start(ones[:], ones_d[:])

            # ---------------- Phase P: projections (+ RoPE for q/k) ----------
            with (
                tc.tile_pool(name="xt", bufs=1) as xtp,
                tc.tile_pool(name="tbl", bufs=1) as tblp,
                tc.tile_pool(name="whd", bufs=2) as whdp,
                tc.tile_pool(name="wvt", bufs=3) as wvtp,
                tc.tile_pool(name="rtmp", bufs=3) as rtmpp,
                tc.tile_pool(name="rot", bufs=3) as rotp,
                tc.tile_pool(name="vev", bufs=3) as vevp,
                tc.tile_pool(name="pps", bufs=3, space="PSUM") as pps,
                tc.tile_pool(name="vps", bufs=1, space="PSUM") as vps,
            ):
                def load_whd(w_dram, h):
                    whd = whdp.tile([128, DC, 128], F32R, tag="whd", name="whd")
                    src = w_dram[:, h * 128:(h + 1) * 128].rearrange(
                        "(d p) e -> p d e", p=128)
                    nc.sync.dma_start(whd[:], src)
                    return whd

                whd0 = load_whd(wk_d, 0)
                xt = []
                for d in range(DC):
                    t = xtp.tile([128, S], F32R, tag=f"xt{d}")
                    nc.sync.dma_start(t[:], xt_d[d * 128:(d + 1) * 128, :])
                    xt.append(t)
                ropec = tblp.tile([128, S], F32, tag="ropec")
                nc.sync.dma_start(ropec[:], ropec_d[:])
                ropes2 = tblp.tile([128, S], F32, tag="ropes2")
                nc.sync.dma_start(ropes2[:], ropes2_d[:])

                def project_rope(w_dram, h, dst, whd=None):
                    """Project one q/k head tile [128, S] with RoPE, spill to dst[h]."""
                    if whd is None:
                        whd = load_whd(w_dram, h)
                    for s in range(SB):
                        sl = slice(s * SBW, (s + 1) * SBW)
                        ps = pps.tile([128, SBW], F32, tag="proj")
                        for d in range(DC):
                            nc.tensor.matmul(ps[:], whd[:, d, :], xt[d][:, sl],
                                             start=(d == 0), stop=(d == DC - 1))
                        # RoPE: rot = ps*C + swap64(ps)*S2  (swap via psum read
                        # at partition offset 64 -- legal since in0 is PSUM)
                        m1 = rtmpp.tile([128, SBW], F32, tag="m1")
                        nc.vector.tensor_mul(m1[:], ps[:], ropec[:, sl])
                        m2 = rtmpp.tile([128, SBW], F32, tag="m2")
                        nc.vector.tensor_mul(m2[0:64, :], ps[64:128, :],
                                             ropes2[0:64, sl])
                        nc.vector.tensor_mul(m2[64:128, :], ps[0:64, :],
                                             ropes2[64:128, sl])
                        rot = rotp.tile([128, SBW], F32R, tag="rot")
                        nc.vector.tensor_add(rot[:], m1[:], m2[:])
                        nc.sync.dma_start(dst[h, :, s * SBW:(s + 1) * SBW], rot[:])

                for h in range(HPC):
                    project_rope(wk_d, h, k_d, whd=whd0 if h == 0 else None)
                for h in range(HPC):
                    project_rope(wq_d, h, q_scr)

                # V: natural layout out[s, e] = sum_d x[s, d] Wv[e, d]
                # lhsT = xt tile column slice (stationary), rhs = wv chunk.
                for eb in range(EVB):
                    ebl = slice(eb * EVW, (eb + 1) * EVW)
                    for stg in range(0, ST, 4):
                        n_st = min(4, ST - stg)
                        pss = []
                        for i in range(n_st):
                            vp = vps.tile([128, EVW], F32, tag=f"vp{i}",
                                          name=f"vp{i}")
                            pss.append(vp)
                        for d in range(DC):
                            wvt = wvtp.tile([128, EVW], F32R, tag="wvt")
                            nc.sync.dma_start(
                                wvt[:], wv_d[d * 128:(d + 1) * 128, ebl])
                            for i in range(n_st):
                                st = stg + i
                                nc.tensor.matmul(
                                    pss[i][:], xt[d][:, st * 128:(st + 1) * 128],
                                    wvt[:], start=(d == 0), stop=(d == DC - 1))
                        for i in range(n_st):
                            st = stg + i
                            vt = vevp.tile([128, EVW], F32R, tag="vev")
                            nc.vector.tensor_copy(vt[:], pss[i][:])
                            nc.sync.dma_start(
                                v_d[st * 128:(st + 1) * 128, ebl], vt[:])

            # ---------------- Phase A: causal attention ----------------------
            with (
                tc.tile_pool(name="qkv", bufs=2) as qkvp,
                tc.tile_pool(name="exp", bufs=6) as expp,
                tc.tile_pool(name="bcst", bufs=3) as bcp,
                tc.tile_pool(name="ctxs", bufs=3) as ctxp,
                tc.tile_pool(name="wo", bufs=1) as wop,
                tc.tile_pool(name="ctxsl", bufs=2) as cslp,
                tc.tile_pool(name="osb", bufs=4) as osbp,
                tc.tile_pool(name="qkps", bufs=3, space="PSUM") as qkps,
                tc.tile_pool(name="ctxps", bufs=2, space="PSUM") as ctxps,
                tc.tile_pool(name="denps", bufs=3, space="PSUM") as denps,
            ):
                wo = []
                for h in range(HPC):
                    t = wop.tile([128, D], F32R, tag=f"wo{h}", name=f"wo{h}")
                    nc.sync.dma_start(t[:], wo_d[h * 128:(h + 1) * 128, :])
                    wo.append(t)
                for h in range(HPC):
                    qT = qkvp.tile([128, S], F32R, tag="qT")
                    nc.sync.dma_start(qT[:], q_scr[h])
                    kT = qkvp.tile([128, S], F32R, tag="kT")
                    nc.sync.dma_start(kT[:], k_d[h])
                    vT = qkvp.tile([128, ST, 128], F32R, tag="vT")
                    nc.sync.dma_start(
                        vT[:], v_d[:, h * 128:(h + 1) * 128].rearrange(
                            "(c p) e -> p c e", p=128))
                    for j in range(QB):
                        q0 = j * SBW
                        n_k = KTPB * j + KTPB
                        ctx_ps = ctxps.tile([128, SBW], F32, tag="ctx")
                        den_ps = denps.tile([1, SBW], F32, tag="den")
                        for i in range(n_k):
                            delta = max(0, (i - KTPB * j) * 128)
                            w = SBW - delta
                            s_ps = qkps.tile([128, SBW], F32, tag="qk")
                            nc.tensor.matmul(
                                s_ps[:, 0:w], kT[:, i * 128:(i + 1) * 128],
                                qT[:, q0 + delta:q0 + SBW],
                                start=True, stop=True)
                            e_sb = expp.tile([128, SBW], F32R, tag="exp")
                            nc.scalar.activation(e_sb[:, 0:w], s_ps[:, 0:w],
                                                 EXPF, scale=inv_sqrt_hd)
                            if i >= KTPB * j:
                                # diagonal tile: triangular causal mask on the
                                # first 128 columns of the restricted range
                                nc.vector.tensor_mul(e_sb[:, 0:128],
                                                     e_sb[:, 0:128], maskt[:])
                            nc.tensor.matmul(
                                ctx_ps[:, delta:SBW], vT[:, i, :], e_sb[:, 0:w],
                                start=(i == 0), stop=(i == n_k - 1))
                            nc.tensor.matmul(
                                den_ps[0:1, delta:SBW], ones[:], e_sb[:, 0:w],
                                start=(i == 0), stop=(i == n_k - 1))
                        recip = bcp.tile([1, SBW], F32, tag="recip")
                        nc.vector.reciprocal_approx_fast(recip[:], den_ps[:])
                        bc = bcp.tile([128, SBW], F32, tag="bc")
                        nc.gpsimd.partition_broadcast(bc[:], recip[:])
                        ctx_sb = ctxp.tile([128, SBW], F32R, tag="ctxsb")
                        nc.vector.tensor_mul(ctx_sb[:], ctx_ps[:], bc[:])
                        nc.sync.dma_start(
                            ctx_scr[h, :, q0:q0 + SBW], ctx_sb[:])

                # ---------------- Phase O: output projection -----------------
                for s in range(SB):
                    sl = slice(s * SBW, (s + 1) * SBW)
                    csl = []
                    for h in range(HPC):
                        t = cslp.tile([128, SBW], F32R, tag=f"csl{h}", name=f"csl{h}")
                        nc.sync.dma_start(t[:], ctx_scr[h, :, sl])
                        csl.append(t)
                    for m in range(D // 128):
                        ps = qkps.tile([128, SBW], F32, tag="qk")
                        for h in range(HPC):
                            nc.tensor.matmul(
                                ps[:], wo[h][:, m * 128:(m + 1) * 128], csl[h][:],
                                start=(h == 0), stop=(h == HPC - 1))
                        ot = osbp.tile([128, SBW], F32, tag="osb")
                        nc.vector.tensor_copy(ot[:], ps[:])
                        nc.sync.dma_start(out_d[m * 128:(m + 1) * 128, sl], ot[:])

    nc.compile()
    return nc


def host_inputs(cfg: Cfg, x, rope_cos, rope_sin, qkv_w, out_w):
    """Build the 8 per-core input maps from full inputs."""
    D, S, HPC = cfg.D, cfg.S, cfg.HPC
    deint = np.concatenate([np.arange(0, 128, 2), np.arange(1, 128, 2)])
    ropec = np.concatenate([rope_cos.T, rope_cos.T], axis=0).astype(np.float32)
    ropec = np.ascontiguousarray(ropec)
    ropes2 = np.concatenate([-rope_sin.T, rope_sin.T], axis=0).astype(np.float32)
    ropes2 = np.ascontiguousarray(ropes2)
    kk, qq = np.meshgrid(np.arange(128), np.arange(128), indexing="ij")
    maskt = (kk <= qq).astype(np.float32)

    in_maps = []
    for c in range(N_CORES):
        b, g = c // 2, c % 2
        xt = np.ascontiguousarray(x[b].T)
        rows_q = np.concatenate(
            [(g * HPC + h) * 128 + deint for h in range(HPC)])
        rows_k = D + rows_q
        rows_v = np.concatenate(
            [2 * D + (g * HPC + h) * 128 + np.arange(128) for h in range(HPC)])
        wq = np.ascontiguousarray(qkv_w[rows_q].T)
        wk = np.ascontiguousarray(qkv_w[rows_k].T)
        wv = np.ascontiguousarray(qkv_w[rows_v].T)
        wo = np.ascontiguousarray(out_w[:, g * HPC * 128:(g + 1) * HPC * 128].T)
        in_maps.append({
            "xt": xt, "wq": wq, "wk": wk, "wv": wv, "wo": wo,
            "ropec": ropec, "ropes2": ropes2, "maskt": maskt,
            "ones1": np.ones((128, 1), dtype=np.float32),
        })
    return in_maps


def assemble(cfg: Cfg, results):
    """Gather per-core results into full (out, k, v)."""
    D, S, HPC = cfg.D, cfg.S, cfg.HPC
    B, H = 4, 2 * HPC
    deint = np.concatenate([np.arange(0, 128, 2), np.arange(1, 128, 2)])
    out = np.empty((B, S, D), dtype=np.float32)
    k = np.empty((B, H, S, 128), dtype=np.float32)
    v = np.empty((B, H, S, 128), dtype=np.float32)
    for b in range(B):
        r0, r1 = results[2 * b], results[2 * b + 1]
        out[b] = (r0["out_t"] + r1["out_t"]).T
        for g, r in ((0, r0), (1, r1)):
            for h in range(HPC):
                k[b, g * HPC + h][:, deint] = r["k_out"][h].T
                v[b, g * HPC + h] = r["v_out"][:, h * 128:(h + 1) * 128]
    return out, k, v


_CACHE = {}


def _get_nc(cfg: Cfg):
    key = (cfg.D, cfg.S, cfg.H)
    if key not in _CACHE:
        _CACHE[key] = build(cfg)
    return _CACHE[key]


def run(inputs, cfg: Cfg | None = None, trace=False):
    cfg = cfg or Cfg()
    nc = _get_nc(cfg)
    in_maps = host_inputs(cfg, inputs["x"], inputs["rope_cos"],
                          inputs["rope_sin"], inputs["qkv_w"], inputs["out_w"])
    res = run_bass_kernel_spmd(nc, in_maps, list(range(N_CORES)), trace=trace)
    return assemble(cfg, res.results), res


def kernel(x, rope_cos, rope_sin, qkv_w, out_w):
    (out, k, v), _ = run({"x": np.asarray(x, dtype=np.float32),
                          "rope_cos": np.asarray(rope_cos, dtype=np.float32),
                          "rope_sin": np.asarray(rope_sin, dtype=np.float32),
                          "qkv_w": np.asarray(qkv_w, dtype=np.float32),
                          "out_w": np.asarray(out_w, dtype=np.float32)})
    return out, k, v


# revision 7
# speedup vs baseline: 1.0698x; 1.0243x over previous
"""Causal self-attention (QKV proj + RoPE + causal SDPA + out proj) on 8 TRN2 cores.

Sharding: 8 cores = 4 batches x 2 head-groups (tensor-parallel over heads).
Core c handles batch b=c//2, head group g=c%2 (8 of 16 heads). Each core:
  - QKV.T projection with head_dim on partitions (f32r matmuls, 1 cyc/row)
  - RoPE applied in a de-interleaved basis (even components in partitions 0:64,
    odd in 64:128) so the rotation pair sits at partition offset +64; the
    de-interleave is folded into the host-side weight row ordering.
  - causal attention computed transposed: S.T[k,q] tiles so exp'd scores feed
    the AV matmul directly as rhs (no P transpose); softmax denominator via
    M=1 ones-matmul accumulated in PSUM; normalize via gpsimd
    partition_broadcast of the reciprocal row.
  - out projection accumulated over the 8 local head chunks -> partial out.T.
Host sums the two tensor-parallel partials per batch and reassembles k/v.

Returns (out, k, v) matching the reference: out [B,S,D], k/v [B,H,S,hd]
(k is post-RoPE, v pre-RoPE), all float32.
"""

import sys

if "/opt/trn_rl_repo" not in sys.path:
    sys.path.insert(0, "/opt/trn_rl_repo")

import math

import numpy as np

import concourse.bacc as bacc
import concourse.mybir as mybir
from concourse import tile
from concourse.bass_utils import run_bass_kernel_spmd

F32 = mybir.dt.float32
F32R = mybir.dt.float32r
EXPF = mybir.ActivationFunctionType.Exp

N_CORES = 8


class Cfg:
    def __init__(self, d_model=2048, s=2048, n_heads=16):
        self.D = d_model
        self.S = s
        self.H = n_heads                  # total heads
        self.HD = 128                     # head dim (fixed: masks/rope assume 128)
        self.HPC = n_heads // 2           # heads per core (2-way head TP)
        self.DC = d_model // 128          # contraction chunks
        self.SB = max(1, s // 512)        # 512-wide s blocks
        self.SBW = min(512, s)            # s block width
        self.ST = s // 128                # 128-row s tiles
        self.EV = self.HPC * 128          # v cols per core
        self.EVB = max(1, self.EV // 512) # v col blocks
        self.EVW = min(512, self.EV)      # v col block width


def build(cfg: Cfg):
    """Build the per-core Bass program (identical across cores; data differs)."""
    nc = bacc.Bacc("TRN2", target_bir_lowering=False, debug=False)
    D, S, HPC, DC, SB, SBW, ST = cfg.D, cfg.S, cfg.HPC, cfg.DC, cfg.SB, cfg.SBW, cfg.ST
    EV, EVB, EVW = cfg.EV, cfg.EVB, cfg.EVW
    QB = SB                     # 512-wide q blocks
    KTPB = SBW // 128           # k tiles per q block width

    xt_d = nc.dram_tensor("xt", [D, S], F32R, kind="ExternalInput")
    wq_d = nc.dram_tensor("wq", [D, HPC * 128], F32R, kind="ExternalInput")
    wk_d = nc.dram_tensor("wk", [D, HPC * 128], F32R, kind="ExternalInput")
    wv_d = nc.dram_tensor("wv", [D, EV], F32R, kind="ExternalInput")
    wo_d = nc.dram_tensor("wo", [EV, D], F32R, kind="ExternalInput")
    ropec_d = nc.dram_tensor("ropec", [128, S], F32, kind="ExternalInput")
    ropes2_d = nc.dram_tensor("ropes2", [128, S], F32, kind="ExternalInput")
    maskt_d = nc.dram_tensor("maskt", [128, 128], F32, kind="ExternalInput")
    ones_d = nc.dram_tensor("ones1", [128, 1], F32R, kind="ExternalInput")

    out_d = nc.dram_tensor("out_t", [D, S], F32, kind="ExternalOutput")
    k_d = nc.dram_tensor("k_out", [HPC, 128, S], F32R, kind="ExternalOutput")
    v_d = nc.dram_tensor("v_out", [S, EV], F32R, kind="ExternalOutput")
    q_scr = nc.dram_tensor("q_scr", [HPC, 128, S], F32R)
    ctx_scr = nc.dram_tensor("ctx_scr", [HPC, 128, S], F32R)

    inv_sqrt_hd = 1.0 / math.sqrt(128.0)

    with tile.TileContext(nc) as tc:
        with tc.tile_pool(name="small", bufs=1) as small:
            maskt = small.tile([128, 128], F32)
            nc.sync.dma_start(maskt[:], maskt_d[:])
            ones = small.tile([128, 1], F32R)
            nc.sync.dma_start(ones[:], ones_d[:])

            # ---------------- Phase P: projections (+ RoPE for q/k) ----------
            with (
                tc.tile_pool(name="xt", bufs=1) as xtp,
                tc.tile_pool(name="tbl", bufs=1) as tblp,
                tc.tile_pool(name="whd", bufs=2) as whdp,
                tc.tile_pool(name="wvt", bufs=3) as wvtp,
                tc.tile_pool(name="rtmp", bufs=3) as rtmpp,
                tc.tile_pool(name="rot", bufs=3) as rotp,
                tc.tile_pool(name="vev", bufs=3) as vevp,
                tc.tile_pool(name="pps", bufs=3, space="PSUM") as pps,
                tc.tile_pool(name="vps", bufs=1, space="PSUM") as vps,
            ):
                def load_whd(w_dram, h):
                    whd = whdp.tile([128, DC, 128], F32R, tag="whd", name="whd")
                    src = w_dram[:, h * 128:(h + 1) * 128].rearrange(
                        "(d p) e -> p d e", p=128)
                    nc.sync.dma_start(whd[:], src)
                    return whd

                whd0 = load_whd(wk_d, 0)
                xt = []
                for d in range(DC):
                    t = xtp.tile([128, S], F32R, tag=f"xt{d}")
                    nc.sync.dma_start(t[:], xt_d[d * 128:(d + 1) * 128, :])
                    xt.append(t)
                ropec = tblp.tile([128, S], F32, tag="ropec")
                nc.sync.dma_start(ropec[:], ropec_d[:])
                ropes2 = tblp.tile([128, S], F32, tag="ropes2")
                nc.sync.dma_start(ropes2[:], ropes2_d[:])

                def project_rope(w_dram, h, dst, whd=None):
                    """Project one q/k head tile [128, S] with RoPE, spill to dst[h]."""
                    if whd is None:
                        whd = load_whd(w_dram, h)
                    for s in range(SB):
                        sl = slice(s * SBW, (s + 1) * SBW)
                        ps = pps.tile([128, SBW], F32, tag="proj")
                        for d in range(DC):
                            nc.tensor.matmul(ps[:], whd[:, d, :], xt[d][:, sl],
                                             start=(d == 0), stop=(d == DC - 1))
                        # RoPE: rot = ps*C + swap64(ps)*S2  (swap via psum read
                        # at partition offset 64 -- legal since in0 is PSUM)
                        m1 = rtmpp.tile([128, SBW], F32, tag="m1")
                        nc.vector.tensor_mul(m1[:], ps[:], ropec[:, sl])
                        m2 = rtmpp.tile([128, SBW], F32, tag="m2")
                        nc.vector.tensor_mul(m2[0:64, :], ps[64:128, :],
                                             ropes2[0:64, sl])
                        nc.vector.tensor_mul(m2[64:128, :], ps[0:64, :],
                                             ropes2[64:128, sl])
                        rot = rotp.tile([128, SBW], F32R, tag="rot")
                        nc.vector.tensor_add(rot[:], m1[:], m2[:])
                        nc.sync.dma_start(dst[h, :, s * SBW:(s + 1) * SBW], rot[:])

                for h in range(HPC):
                    project_rope(wk_d, h, k_d, whd=whd0 if h == 0 else None)
                for h in range(HPC):
                    project_rope(wq_d, h, q_scr)

                # V: natural layout out[s, e] = sum_d x[s, d] Wv[e, d]
                # lhsT = xt tile column slice (stationary), rhs = wv chunk.
                for eb in range(EVB):
                    ebl = slice(eb * EVW, (eb + 1) * EVW)
                    for stg in range(0, ST, 4):
                        n_st = min(4, ST - stg)
                        pss = []
                        for i in range(n_st):
                            vp = vps.tile([128, EVW], F32, tag=f"vp{i}",
                                          name=f"vp{i}")
                            pss.append(vp)
                        for d in range(DC):
                            wvt = wvtp.tile([128, EVW], F32R, tag="wvt")
                            nc.sync.dma_start(
                                wvt[:], wv_d[d * 128:(d + 1) * 128, ebl])
                            for i in range(n_st):
                                st = stg + i
                                nc.tensor.matmul(
                                    pss[i][:], xt[d][:, st * 128:(st + 1) * 128],
                                    wvt[:], start=(d == 0), stop=(d == DC - 1))
                        for i in range(n_st):
                            st = stg + i
                            vt = vevp.tile([128, EVW], F32R, tag="vev")
                            nc.vector.tensor_copy(vt[:], pss[i][:])
                            nc.sync.dma_start(
                                v_d[st * 128:(st + 1) * 128, ebl], vt[:])

            # ---------------- Phase A: causal attention ----------------------
            with (
                tc.tile_pool(name="qkv", bufs=2) as qkvp,
                tc.tile_pool(name="exp", bufs=6) as expp,
                tc.tile_pool(name="bcst", bufs=3) as bcp,
                tc.tile_pool(name="ctxs", bufs=3) as ctxp,
                tc.tile_pool(name="wo", bufs=1) as wop,
                tc.tile_pool(name="ctxsl", bufs=2) as cslp,
                tc.tile_pool(name="osb", bufs=4) as osbp,
                tc.tile_pool(name="qkps", bufs=3, space="PSUM") as qkps,
                tc.tile_pool(name="ctxps", bufs=3, space="PSUM") as ctxps,
                tc.tile_pool(name="denps", bufs=2, space="PSUM") as denps,
            ):
                wo = []
                for h in range(HPC):
                    t = wop.tile([128, D], F32R, tag=f"wo{h}", name=f"wo{h}")
                    nc.gpsimd.dma_start(t[:], wo_d[h * 128:(h + 1) * 128, :])
                    wo.append(t)
                for h in range(HPC):
                    qT = qkvp.tile([128, S], F32R, tag="qT")
                    nc.gpsimd.dma_start(qT[:], q_scr[h])
                    kT = qkvp.tile([128, S], F32R, tag="kT")
                    nc.gpsimd.dma_start(kT[:], k_d[h])
                    vT = qkvp.tile([128, ST, 128], F32R, tag="vT")
                    nc.gpsimd.dma_start(
                        vT[:], v_d[:, h * 128:(h + 1) * 128].rearrange(
                            "(c p) e -> p c e", p=128))
                    for j in range(QB):
                        q0 = j * SBW
                        n_k = KTPB * j + KTPB
                        ctx_ps = ctxps.tile([128, SBW], F32, tag="ctx")
                        den_ps = denps.tile([1, SBW], F32, tag="den")
                        for i in range(n_k):
                            delta = max(0, (i - KTPB * j) * 128)
                            w = SBW - delta
                            s_ps = qkps.tile([128, SBW], F32, tag="qk")
                            nc.tensor.matmul(
                                s_ps[:, 0:w], kT[:, i * 128:(i + 1) * 128],
                                qT[:, q0 + delta:q0 + SBW],
                                start=True, stop=True)
                            e_sb = expp.tile([128, SBW], F32R, tag="exp")
                            nc.scalar.activation(e_sb[:, 0:w], s_ps[:, 0:w],
                                                 EXPF, scale=inv_sqrt_hd)
                            if i >= KTPB * j:
                                # diagonal tile: triangular causal mask on the
                                # first 128 columns of the restricted range
                                nc.vector.tensor_mul(e_sb[:, 0:128],
                                                     e_sb[:, 0:128], maskt[:])
                            nc.tensor.matmul(
                                ctx_ps[:, delta:SBW], vT[:, i, :], e_sb[:, 0:w],
                                start=(i == 0), stop=(i == n_k - 1))
                            nc.tensor.matmul(
                                den_ps[0:1, delta:SBW], ones[:], e_sb[:, 0:w],
                                start=(i == 0), stop=(i == n_k - 1))
                        recip = bcp.tile([1, SBW], F32, tag="recip")
                        nc.vector.reciprocal_approx_fast(recip[:], den_ps[:])
                        bc = bcp.tile([128, SBW], F32, tag="bc")
                        nc.gpsimd.partition_broadcast(bc[:], recip[:])
                        ctx_sb = ctxp.tile([128, SBW], F32R, tag="ctxsb")
                        nc.vector.tensor_mul(ctx_sb[:], ctx_ps[:], bc[:])
                        nc.sync.dma_start(
                            ctx_scr[h, :, q0:q0 + SBW], ctx_sb[:])

                # ---------------- Phase O: output projection -----------------
                for s in range(SB):
                    sl = slice(s * SBW, (s + 1) * SBW)
                    csl = []
                    for h in range(HPC):
                        t = cslp.tile([128, SBW], F32R, tag=f"csl{h}", name=f"csl{h}")
                        nc.gpsimd.dma_start(t[:], ctx_scr[h, :, sl])
                        csl.append(t)
                    for m in range(D // 128):
                        ps = qkps.tile([128, SBW], F32, tag="qk")
                        for h in range(HPC):
                            nc.tensor.matmul(
                                ps[:], wo[h][:, m * 128:(m + 1) * 128], csl[h][:],
                                start=(h == 0), stop=(h == HPC - 1))
                        ot = osbp.tile([128, SBW], F32, tag="osb")
                        nc.vector.tensor_copy(ot[:], ps[:])
                        nc.sync.dma_start(out_d[m * 128:(m + 1) * 128, sl], ot[:])

    nc.compile()
    return nc


def host_inputs(cfg: Cfg, x, rope_cos, rope_sin, qkv_w, out_w):
    """Build the 8 per-core input maps from full inputs."""
    D, S, HPC = cfg.D, cfg.S, cfg.HPC
    deint = np.concatenate([np.arange(0, 128, 2), np.arange(1, 128, 2)])
    ropec = np.concatenate([rope_cos.T, rope_cos.T], axis=0).astype(np.float32)
    ropec = np.ascontiguousarray(ropec)
    ropes2 = np.concatenate([-rope_sin.T, rope_sin.T], axis=0).astype(np.float32)
    ropes2 = np.ascontiguousarray(ropes2)
    kk, qq = np.meshgrid(np.arange(128), np.arange(128), indexing="ij")
    maskt = (kk <= qq).astype(np.float32)

    in_maps = []
    for c in range(N_CORES):
        b, g = c // 2, c % 2
        xt = np.ascontiguousarray(x[b].T)
        rows_q = np.concatenate(
            [(g * HPC + h) * 128 + deint for h in range(HPC)])
        rows_k = D + rows_q
        rows_v = np.concatenate(
            [2 * D + (g * HPC + h) * 128 + np.arange(128) for h in range(HPC)])
        wq = np.ascontiguousarray(qkv_w[rows_q].T)
        wk = np.ascontiguousarray(qkv_w[rows_k].T)
        wv = np.ascontiguousarray(qkv_w[rows_v].T)
        wo = np.ascontiguousarray(out_w[:, g * HPC * 128:(g + 1) * HPC * 128].T)
        in_maps.append({
            "xt": xt, "wq": wq, "wk": wk, "wv": wv, "wo": wo,
            "ropec": ropec, "ropes2": ropes2, "maskt": maskt,
            "ones1": np.ones((128, 1), dtype=np.float32),
        })
    return in_maps


def assemble(cfg: Cfg, results):
    """Gather per-core results into full (out, k, v)."""
    D, S, HPC = cfg.D, cfg.S, cfg.HPC
    B, H = 4, 2 * HPC
    deint = np.concatenate([np.arange(0, 128, 2), np.arange(1, 128, 2)])
    out = np.empty((B, S, D), dtype=np.float32)
    k = np.empty((B, H, S, 128), dtype=np.float32)
    v = np.empty((B, H, S, 128), dtype=np.float32)
    for b in range(B):
        r0, r1 = results[2 * b], results[2 * b + 1]
        out[b] = (r0["out_t"] + r1["out_t"]).T
        for g, r in ((0, r0), (1, r1)):
            for h in range(HPC):
                k[b, g * HPC + h][:, deint] = r["k_out"][h].T
                v[b, g * HPC + h] = r["v_out"][:, h * 128:(h + 1) * 128]
    return out, k, v


_CACHE = {}


def _get_nc(cfg: Cfg):
    key = (cfg.D, cfg.S, cfg.H)
    if key not in _CACHE:
        _CACHE[key] = build(cfg)
    return _CACHE[key]


def run(inputs, cfg: Cfg | None = None, trace=False):
    cfg = cfg or Cfg()
    nc = _get_nc(cfg)
    in_maps = host_inputs(cfg, inputs["x"], inputs["rope_cos"],
                          inputs["rope_sin"], inputs["qkv_w"], inputs["out_w"])
    res = run_bass_kernel_spmd(nc, in_maps, list(range(N_CORES)), trace=trace)
    return assemble(cfg, res.results), res


def kernel(x, rope_cos, rope_sin, qkv_w, out_w):
    (out, k, v), _ = run({"x": np.asarray(x, dtype=np.float32),
                          "rope_cos": np.asarray(rope_cos, dtype=np.float32),
                          "rope_sin": np.asarray(rope_sin, dtype=np.float32),
                          "qkv_w": np.asarray(qkv_w, dtype=np.float32),
                          "out_w": np.asarray(out_w, dtype=np.float32)})
    return out, k, v


# revision 8
# speedup vs baseline: 1.1437x; 1.0690x over previous
"""Causal self-attention (QKV proj + RoPE + causal SDPA + out proj) on 8 TRN2 cores.

Sharding: 8 cores = 4 batches x 2 head-groups (tensor-parallel over heads).
Core c handles batch b=c//2, head group g=c%2 (8 of 16 heads). Each core:
  - QKV.T projection with head_dim on partitions (f32r matmuls, 1 cyc/row)
  - RoPE applied in a de-interleaved basis (even components in partitions 0:64,
    odd in 64:128) so the rotation pair sits at partition offset +64; the
    de-interleave is folded into the host-side weight row ordering.
  - causal attention computed transposed: S.T[k,q] tiles so exp'd scores feed
    the AV matmul directly as rhs (no P transpose); softmax denominator via
    M=1 ones-matmul accumulated in PSUM; normalize via gpsimd
    partition_broadcast of the reciprocal row.
  - out projection accumulated over the 8 local head chunks -> partial out.T.
Host sums the two tensor-parallel partials per batch and reassembles k/v.

Returns (out, k, v) matching the reference: out [B,S,D], k/v [B,H,S,hd]
(k is post-RoPE, v pre-RoPE), all float32.
"""

import sys

if "/opt/trn_rl_repo" not in sys.path:
    sys.path.insert(0, "/opt/trn_rl_repo")

import math

import numpy as np

import concourse.bacc as bacc
import concourse.mybir as mybir
from concourse import tile
from concourse.bass_utils import run_bass_kernel_spmd

F32 = mybir.dt.float32
F32R = mybir.dt.float32r
EXPF = mybir.ActivationFunctionType.Exp

N_CORES = 8


class Cfg:
    def __init__(self, d_model=2048, s=2048, n_heads=16):
        self.D = d_model
        self.S = s
        self.H = n_heads                  # total heads
        self.HD = 128                     # head dim (fixed: masks/rope assume 128)
        self.HPC = n_heads // 2           # heads per core (2-way head TP)
        self.DC = d_model // 128          # contraction chunks
        self.SB = max(1, s // 512)        # 512-wide s blocks
        self.SBW = min(512, s)            # s block width
        self.ST = s // 128                # 128-row s tiles
        self.EV = self.HPC * 128          # v cols per core
        self.EVB = max(1, self.EV // 512) # v col blocks
        self.EVW = min(512, self.EV)      # v col block width


def build(cfg: Cfg):
    """Build the per-core Bass program (identical across cores; data differs)."""
    nc = bacc.Bacc("TRN2", target_bir_lowering=False, debug=False)
    D, S, HPC, DC, SB, SBW, ST = cfg.D, cfg.S, cfg.HPC, cfg.DC, cfg.SB, cfg.SBW, cfg.ST
    EV, EVB, EVW = cfg.EV, cfg.EVB, cfg.EVW
    QB = SB                     # 512-wide q blocks
    KTPB = SBW // 128           # k tiles per q block width

    xt_d = nc.dram_tensor("xt", [D, S], F32R, kind="ExternalInput")
    wq_d = nc.dram_tensor("wq", [D, HPC * 128], F32R, kind="ExternalInput")
    wk_d = nc.dram_tensor("wk", [D, HPC * 128], F32R, kind="ExternalInput")
    wv_d = nc.dram_tensor("wv", [D, EV], F32R, kind="ExternalInput")
    wo_d = nc.dram_tensor("wo", [EV, D], F32R, kind="ExternalInput")
    ropec_d = nc.dram_tensor("ropec", [128, S], F32, kind="ExternalInput")
    ropes2_d = nc.dram_tensor("ropes2", [128, S], F32, kind="ExternalInput")
    maskt_d = nc.dram_tensor("maskt", [128, 128], F32, kind="ExternalInput")
    ones_d = nc.dram_tensor("ones1", [128, 1], F32R, kind="ExternalInput")

    out_d = nc.dram_tensor("out_t", [D, S], F32, kind="ExternalOutput")
    k_d = nc.dram_tensor("k_out", [HPC, 128, S], F32R, kind="ExternalOutput")
    v_d = nc.dram_tensor("v_out", [S, EV], F32R, kind="ExternalOutput")
    q_scr = nc.dram_tensor("q_scr", [HPC, 128, S], F32R)
    ctx_scr = nc.dram_tensor("ctx_scr", [HPC, 128, S], F32R)

    inv_sqrt_hd = 1.0 / math.sqrt(128.0)

    with tile.TileContext(nc) as tc:
        with tc.tile_pool(name="small", bufs=1) as small:
            maskt = small.tile([128, 128], F32)
            nc.sync.dma_start(maskt[:], maskt_d[:])
            ones = small.tile([128, 1], F32R)
            nc.sync.dma_start(ones[:], ones_d[:])

            # ---------------- Phase P: projections (+ RoPE for q/k) ----------
            with (
                tc.tile_pool(name="xt", bufs=1) as xtp,
                tc.tile_pool(name="tbl", bufs=1) as tblp,
                tc.tile_pool(name="whd", bufs=2) as whdp,
                tc.tile_pool(name="wvt", bufs=3) as wvtp,
                tc.tile_pool(name="rtmp", bufs=3) as rtmpp,
                tc.tile_pool(name="rot", bufs=3) as rotp,
                tc.tile_pool(name="vev", bufs=3) as vevp,
                tc.tile_pool(name="pps", bufs=4, space="PSUM") as pps,
                tc.tile_pool(name="vps", bufs=1, space="PSUM") as vps,
            ):
                def load_whd(w_dram, h):
                    whd = whdp.tile([128, DC, 128], F32R, tag="whd", name="whd")
                    src = w_dram[:, h * 128:(h + 1) * 128].rearrange(
                        "(d p) e -> p d e", p=128)
                    nc.gpsimd.dma_start(whd[:], src)
                    return whd

                whd0 = load_whd(wk_d, 0)
                xt = []
                for d in range(DC):
                    t = xtp.tile([128, S], F32R, tag=f"xt{d}")
                    nc.sync.dma_start(t[:], xt_d[d * 128:(d + 1) * 128, :])
                    xt.append(t)
                ropec = tblp.tile([128, S], F32, tag="ropec")
                nc.sync.dma_start(ropec[:], ropec_d[:])
                ropes2 = tblp.tile([128, S], F32, tag="ropes2")
                nc.sync.dma_start(ropes2[:], ropes2_d[:])

                def project_rope(w_dram, h, dst, whd=None):
                    """Project one q/k head tile [128, S] with RoPE, spill to dst[h]."""
                    if whd is None:
                        whd = load_whd(w_dram, h)
                    for s in range(SB):
                        sl = slice(s * SBW, (s + 1) * SBW)
                        ps = pps.tile([128, SBW], F32, tag="proj")
                        for d in range(DC):
                            nc.tensor.matmul(ps[:], whd[:, d, :], xt[d][:, sl],
                                             start=(d == 0), stop=(d == DC - 1))
                        # RoPE: rot = ps*C + swap64(ps)*S2  (swap via psum read
                        # at partition offset 64 -- legal since in0 is PSUM)
                        m1 = rtmpp.tile([128, SBW], F32, tag="m1")
                        nc.vector.tensor_mul(m1[:], ps[:], ropec[:, sl])
                        m2 = rtmpp.tile([128, SBW], F32, tag="m2")
                        nc.vector.tensor_mul(m2[0:64, :], ps[64:128, :],
                                             ropes2[0:64, sl])
                        nc.vector.tensor_mul(m2[64:128, :], ps[0:64, :],
                                             ropes2[64:128, sl])
                        rot = rotp.tile([128, SBW], F32R, tag="rot")
                        nc.vector.tensor_add(rot[:], m1[:], m2[:])
                        nc.sync.dma_start(dst[h, :, s * SBW:(s + 1) * SBW], rot[:])

                for h in range(HPC):
                    project_rope(wk_d, h, k_d, whd=whd0 if h == 0 else None)
                for h in range(HPC):
                    project_rope(wq_d, h, q_scr)

                # V: natural layout out[s, e] = sum_d x[s, d] Wv[e, d]
                # lhsT = xt tile column slice (stationary), rhs = wv chunk.
                for eb in range(EVB):
                    ebl = slice(eb * EVW, (eb + 1) * EVW)
                    for stg in range(0, ST, 4):
                        n_st = min(4, ST - stg)
                        pss = []
                        for i in range(n_st):
                            vp = vps.tile([128, EVW], F32, tag=f"vp{i}",
                                          name=f"vp{i}")
                            pss.append(vp)
                        for d in range(DC):
                            wvt = wvtp.tile([128, EVW], F32R, tag="wvt")
                            nc.sync.dma_start(
                                wvt[:], wv_d[d * 128:(d + 1) * 128, ebl])
                            for i in range(n_st):
                                st = stg + i
                                nc.tensor.matmul(
                                    pss[i][:], xt[d][:, st * 128:(st + 1) * 128],
                                    wvt[:], start=(d == 0), stop=(d == DC - 1))
                        for i in range(n_st):
                            st = stg + i
                            vt = vevp.tile([128, EVW], F32R, tag="vev")
                            nc.vector.tensor_copy(vt[:], pss[i][:])
                            nc.sync.dma_start(
                                v_d[st * 128:(st + 1) * 128, ebl], vt[:])

            # ---------------- Phase A: causal attention ----------------------
            with (
                tc.tile_pool(name="qkv", bufs=2) as qkvp,
                tc.tile_pool(name="exp", bufs=6) as expp,
                tc.tile_pool(name="bcst", bufs=3) as bcp,
                tc.tile_pool(name="ctxs", bufs=3) as ctxp,
                tc.tile_pool(name="wo", bufs=1) as wop,
                tc.tile_pool(name="ctxsl", bufs=2) as cslp,
                tc.tile_pool(name="osb", bufs=4) as osbp,
                tc.tile_pool(name="qkps", bufs=3, space="PSUM") as qkps,
                tc.tile_pool(name="ctxps", bufs=3, space="PSUM") as ctxps,
                tc.tile_pool(name="denps", bufs=2, space="PSUM") as denps,
            ):
                wo = []

                def load_wo():
                    for hh in range(HPC):
                        t = wop.tile([128, D], F32R, tag=f"wo{hh}", name=f"wo{hh}")
                        nc.gpsimd.dma_start(t[:], wo_d[hh * 128:(hh + 1) * 128, :])
                        wo.append(t)

                for h in range(HPC):
                    if h == min(2, HPC - 1):
                        load_wo()
                    qT = qkvp.tile([128, S], F32R, tag="qT")
                    nc.gpsimd.dma_start(qT[:], q_scr[h])
                    kT = qkvp.tile([128, S], F32R, tag="kT")
                    nc.gpsimd.dma_start(kT[:], k_d[h])
                    vT = qkvp.tile([128, ST, 128], F32R, tag="vT")
                    nc.gpsimd.dma_start(
                        vT[:], v_d[:, h * 128:(h + 1) * 128].rearrange(
                            "(c p) e -> p c e", p=128))
                    for j in range(QB):
                        q0 = j * SBW
                        n_k = KTPB * j + KTPB
                        ctx_ps = ctxps.tile([128, SBW], F32, tag="ctx")
                        den_ps = denps.tile([1, SBW], F32, tag="den")
                        for i in range(n_k):
                            delta = max(0, (i - KTPB * j) * 128)
                            w = SBW - delta
                            s_ps = qkps.tile([128, SBW], F32, tag="qk")
                            nc.tensor.matmul(
                                s_ps[:, 0:w], kT[:, i * 128:(i + 1) * 128],
                                qT[:, q0 + delta:q0 + SBW],
                                start=True, stop=True)
                            e_sb = expp.tile([128, SBW], F32R, tag="exp")
                            nc.scalar.activation(e_sb[:, 0:w], s_ps[:, 0:w],
                                                 EXPF, scale=inv_sqrt_hd)
                            if i >= KTPB * j:
                                # diagonal tile: triangular causal mask on the
                                # first 128 columns of the restricted range
                                nc.vector.tensor_mul(e_sb[:, 0:128],
                                                     e_sb[:, 0:128], maskt[:])
                            nc.tensor.matmul(
                                ctx_ps[:, delta:SBW], vT[:, i, :], e_sb[:, 0:w],
                                start=(i == 0), stop=(i == n_k - 1))
                            nc.tensor.matmul(
                                den_ps[0:1, delta:SBW], ones[:], e_sb[:, 0:w],
                                start=(i == 0), stop=(i == n_k - 1))
                        recip = bcp.tile([1, SBW], F32, tag="recip")
                        nc.vector.reciprocal_approx_fast(recip[:], den_ps[:])
                        bc = bcp.tile([128, SBW], F32, tag="bc")
                        nc.gpsimd.partition_broadcast(bc[:], recip[:])
                        ctx_sb = ctxp.tile([128, SBW], F32R, tag="ctxsb")
                        nc.vector.tensor_mul(ctx_sb[:], ctx_ps[:], bc[:])
                        nc.sync.dma_start(
                            ctx_scr[h, :, q0:q0 + SBW], ctx_sb[:])

                # ---------------- Phase O: output projection -----------------
                for s in range(SB):
                    sl = slice(s * SBW, (s + 1) * SBW)
                    csl = []
                    for h in range(HPC):
                        t = cslp.tile([128, SBW], F32R, tag=f"csl{h}", name=f"csl{h}")
                        nc.gpsimd.dma_start(t[:], ctx_scr[h, :, sl])
                        csl.append(t)
                    for m in range(D // 128):
                        ps = qkps.tile([128, SBW], F32, tag="qk")
                        for h in range(HPC):
                            nc.tensor.matmul(
                                ps[:], wo[h][:, m * 128:(m + 1) * 128], csl[h][:],
                                start=(h == 0), stop=(h == HPC - 1))
                        ot = osbp.tile([128, SBW], F32, tag="osb")
                        nc.vector.tensor_copy(ot[:], ps[:])
                        nc.sync.dma_start(out_d[m * 128:(m + 1) * 128, sl], ot[:])

    nc.compile()
    return nc


def host_inputs(cfg: Cfg, x, rope_cos, rope_sin, qkv_w, out_w):
    """Build the 8 per-core input maps from full inputs."""
    D, S, HPC = cfg.D, cfg.S, cfg.HPC
    deint = np.concatenate([np.arange(0, 128, 2), np.arange(1, 128, 2)])
    ropec = np.concatenate([rope_cos.T, rope_cos.T], axis=0).astype(np.float32)
    ropec = np.ascontiguousarray(ropec)
    ropes2 = np.concatenate([-rope_sin.T, rope_sin.T], axis=0).astype(np.float32)
    ropes2 = np.ascontiguousarray(ropes2)
    kk, qq = np.meshgrid(np.arange(128), np.arange(128), indexing="ij")
    maskt = (kk <= qq).astype(np.float32)

    in_maps = []
    for c in range(N_CORES):
        b, g = c // 2, c % 2
        xt = np.ascontiguousarray(x[b].T)
        rows_q = np.concatenate(
            [(g * HPC + h) * 128 + deint for h in range(HPC)])
        rows_k = D + rows_q
        rows_v = np.concatenate(
            [2 * D + (g * HPC + h) * 128 + np.arange(128) for h in range(HPC)])
        wq = np.ascontiguousarray(qkv_w[rows_q].T)
        wk = np.ascontiguousarray(qkv_w[rows_k].T)
        wv = np.ascontiguousarray(qkv_w[rows_v].T)
        wo = np.ascontiguousarray(out_w[:, g * HPC * 128:(g + 1) * HPC * 128].T)
        in_maps.append({
            "xt": xt, "wq": wq, "wk": wk, "wv": wv, "wo": wo,
            "ropec": ropec, "ropes2": ropes2, "maskt": maskt,
            "ones1": np.ones((128, 1), dtype=np.float32),
        })
    return in_maps


def assemble(cfg: Cfg, results):
    """Gather per-core results into full (out, k, v)."""
    D, S, HPC = cfg.D, cfg.S, cfg.HPC
    B, H = 4, 2 * HPC
    deint = np.concatenate([np.arange(0, 128, 2), np.arange(1, 128, 2)])
    out = np.empty((B, S, D), dtype=np.float32)
    k = np.empty((B, H, S, 128), dtype=np.float32)
    v = np.empty((B, H, S, 128), dtype=np.float32)
    for b in range(B):
        r0, r1 = results[2 * b], results[2 * b + 1]
        out[b] = (r0["out_t"] + r1["out_t"]).T
        for g, r in ((0, r0), (1, r1)):
            for h in range(HPC):
                k[b, g * HPC + h][:, deint] = r["k_out"][h].T
                v[b, g * HPC + h] = r["v_out"][:, h * 128:(h + 1) * 128]
    return out, k, v


_CACHE = {}


def _get_nc(cfg: Cfg):
    key = (cfg.D, cfg.S, cfg.H)
    if key not in _CACHE:
        _CACHE[key] = build(cfg)
    return _CACHE[key]


def run(inputs, cfg: Cfg | None = None, trace=False):
    cfg = cfg or Cfg()
    nc = _get_nc(cfg)
    in_maps = host_inputs(cfg, inputs["x"], inputs["rope_cos"],
                          inputs["rope_sin"], inputs["qkv_w"], inputs["out_w"])
    res = run_bass_kernel_spmd(nc, in_maps, list(range(N_CORES)), trace=trace)
    return assemble(cfg, res.results), res


def kernel(x, rope_cos, rope_sin, qkv_w, out_w):
    (out, k, v), _ = run({"x": np.asarray(x, dtype=np.float32),
                          "rope_cos": np.asarray(rope_cos, dtype=np.float32),
                          "rope_sin": np.asarray(rope_sin, dtype=np.float32),
                          "qkv_w": np.asarray(qkv_w, dtype=np.float32),
                          "out_w": np.asarray(out_w, dtype=np.float32)})
    return out, k, v
